# revision 13
# baseline (speedup 1.0000x reference)
"""CrossViewFusion kernel for Trainium2, 8 NeuronCores, batch-data-parallel.

Computes, per batch b:
    q   = l2norm(global_query[b])                 (cancels under min-max, skipped)
    vn  = value[b] / max(||value[b,:,w,h]||_2, eps)   (norm over channel dim)
    s   = einsum('d,dwh->wh', q, vn)
    attn= (s - min s) / (max s - min s)
    ctx = attn * vn
Returns (context [B,D,W,H], attn [B,1,W,H]) like the reference module.
"""

import sys

import numpy as np

if "/opt/trn_rl_repo" not in sys.path:
    sys.path.insert(0, "/opt/trn_rl_repo")

from contextlib import ExitStack

import concourse.bacc as bacc
import concourse.bass_isa as bass_isa
import concourse.mybir as mybir
import concourse.tile as tile
from concourse.bass_utils import run_bass_kernel_spmd

B, D, W, H = 32, 512, 64, 64
WH = W * H              # 4096
NCORES = 8
BL = B // NCORES        # 4 batches per core
P = 128
CB = D // P             # 4 channel blocks
QT = 4                  # quarters of the spatial dim per accumulation round
QW = WH // QT           # 1024
CW = 512                # matmul free-dim chunk (one PSUM bank of fp32)
NCHUNK = WH // CW       # 8
F32 = mybir.dt.float32
F32R = mybir.dt.float32r
EPS = 1e-12

_CACHE = {}


def _build():
    nc = bacc.Bacc("TRN2", target_bir_lowering=False, debug=False)

    v_h = nc.dram_tensor("v", [BL, D, WH], F32, kind="ExternalInput")
    qlc_h = nc.dram_tensor("qlc", [P, 2 * BL * CB], F32, kind="ExternalInput")
    selc_h = nc.dram_tensor("selc", [NCHUNK, NCHUNK * P], F32, kind="ExternalInput")
    ctx_h = nc.dram_tensor("ctx", [BL, D, WH], F32, kind="ExternalOutput")
    attn_h = nc.dram_tensor("attn", [BL, WH], F32, kind="ExternalOutput")

    with tile.TileContext(nc) as tc:
        with ExitStack() as ex:
            v_pool = ex.enter_context(tc.tile_pool(name="v", bufs=2 * CB))
            vsq_pool = ex.enter_context(tc.tile_pool(name="vsq", bufs=6))
            sums_pool = ex.enter_context(tc.tile_pool(name="sums", bufs=3))
            nm_pool = ex.enter_context(tc.tile_pool(name="nm", bufs=2))
            sm_pool = ex.enter_context(tc.tile_pool(name="sm", bufs=2))
            srow_pool = ex.enter_context(tc.tile_pool(name="srow", bufs=2))
            cpool = ex.enter_context(tc.tile_pool(name="const", bufs=1))
            ss_pool = ex.enter_context(
                tc.tile_pool(name="ss", bufs=2, space="PSUM")
            )
            sc_pool = ex.enter_context(
                tc.tile_pool(name="sc", bufs=2, space="PSUM")
            )
            bc_pool = ex.enter_context(
                tc.tile_pool(name="bc", bufs=2, space="PSUM")
            )

            # qL holds [ones | q_block] column pairs for every (batch, channel
            # block): col 2j = 1.0, col 2j+1 = q[b, cb*128:(cb+1)*128].
            qL = cpool.tile([P, 2 * BL * CB], F32, tag="qL")
            # sel[k, n*128:(n+1)*128] = (k == n): one-hot lhsT blocks used to
            # broadcast row n of an [8, 512] tile to all 128 partitions via PE.
            sel = cpool.tile([NCHUNK, NCHUNK * P], F32, tag="sel")
            nc.gpsimd.dma_start(out=qL[:].bitcast(F32R), in_=qlc_h[:, :])
            nc.sync.dma_start(out=sel[:], in_=selc_h[:, :])

            for b in range(BL):
                vts = []
                for cb in range(CB):
                    vt = v_pool.tile([P, WH], F32, tag="v")
                    nc.gpsimd.dma_start(
                        out=vt[:].bitcast(F32R),
                        in_=v_h[b, cb * P : (cb + 1) * P, :],
                    )
                    vts.append(vt)

                # nm[:, 0:32] = sumsq, nm[:, 32:64] = raw score, laid out as
                # pos = p*32 + f  (partition-major).
                nm = nm_pool.tile([P, 64], F32, tag="nm")
                for qtr in range(QT):
                    vsqs = []
                    for cb in range(CB):
                        vsq = vsq_pool.tile([P, QW], F32, tag="vsq")
                        nc.scalar.square(
                            out=vsq[:].bitcast(F32R),
                            in_=vts[cb][:, qtr * QW : (qtr + 1) * QW],
                        )
                        vsqs.append(vsq)
                    for h in range(QW // CW):
                        # global 512-chunk index; covers nm partitions
                        # [c*16, (c+1)*16) in the pos = p*32 + f layout
                        c = qtr * (QW // CW) + h
                        sl = slice(h * CW, (h + 1) * CW)
                        gsl = slice(qtr * QW + h * CW, qtr * QW + (h + 1) * CW)
                        ss = ss_pool.tile([1, CW], F32, tag="ss")
                        sc = sc_pool.tile([1, CW], F32, tag="sc")
                        for cb in range(CB):
                            j = b * CB + cb
                            # sumsq chain: lhsT = ones column
                            nc.tensor.matmul(
                                ss[0:1, :],
                                qL[:, 2 * j : 2 * j + 1].bitcast(F32R),
                                vsqs[cb][:, sl].bitcast(F32R),
                                start=(cb == 0),
                                stop=(cb == CB - 1),
                            )
                            # raw score chain: lhsT = q column
                            nc.tensor.matmul(
                                sc[0:1, :],
                                qL[:, 2 * j + 1 : 2 * j + 2].bitcast(F32R),
                                vts[cb][:, gsl].bitcast(F32R),
                                start=(cb == 0),
                                stop=(cb == CB - 1),
                            )
                        ss_sb = sums_pool.tile([1, CW], F32, tag="ss_sb")
                        sc_sb = sums_pool.tile([1, CW], F32, tag="sc_sb")
                        nc.scalar.copy(out=ss_sb[:], in_=ss[:])
                        nc.vector.tensor_copy(out=sc_sb[:], in_=sc[:])
                        psl = slice(c * 16, (c + 1) * 16)
                        nc.sync.dma_start(
                            out=nm[psl, 0:32],
                            in_=ss_sb[:].rearrange("o (p f) -> o p f", f=32),
                        )
                        nc.sync.dma_start(
                            out=nm[psl, 32:64],
                            in_=sc_sb[:].rearrange("o (p f) -> o p f", f=32),
                        )

                # per-position scalar chain in [128, 32] layout
                vscale = sm_pool.tile([P, 32], F32, tag="vscale")
                nc.scalar.sqrt(out=vscale[:], in_=nm[:, 0:32])
                nc.vector.tensor_scalar_max(
                    out=vscale[:], in0=vscale[:], scalar1=EPS
                )
                nc.vector.reciprocal(out=vscale[:], in_=vscale[:])
                score = sm_pool.tile([P, 32], F32, tag="score")
                nc.vector.tensor_mul(
                    out=score[:], in0=nm[:, 32:64], in1=vscale[:]
                )
                # per-partition max / -min pairs, then a tiny DMA repack to a
                # single row so DVE can finish the cross-partition reduction
                # (keeps gpsimd free for the cast-loads).
                mm2 = sm_pool.tile([P, 2], F32, tag="mm2")
                nc.vector.tensor_reduce(
                    out=mm2[:, 0:1],
                    in_=score[:],
                    axis=mybir.AxisListType.X,
                    op=mybir.AluOpType.max,
                )
                nc.vector.tensor_reduce(
                    out=mm2[:, 1:2],
                    in_=score[:],
                    axis=mybir.AxisListType.X,
                    op=mybir.AluOpType.min,
                )
                nc.vector.tensor_scalar_mul(
                    out=mm2[:, 1:2], in0=mm2[:, 1:2], scalar1=-1.0
                )
                r256 = sm_pool.tile([1, 2 * P], F32, tag="r256")
                nc.sync.dma_start(
                    out=r256[0:1, :].rearrange("o (p c) -> o p c", c=2),
                    in_=mm2[:, :],
                )
                r2 = sm_pool.tile([1, 2], F32, tag="r2")  # [mx, -mn]
                nc.vector.tensor_reduce(
                    out=r2[0:1, :],
                    in_=r256[0:1, :].rearrange("o (p c) -> o c p", c=2),
                    axis=mybir.AxisListType.X,
                    op=mybir.AluOpType.max,
                )
                d = sm_pool.tile([1, 1], F32, tag="d")  # 1/(mx - mn)
                nc.vector.tensor_add(
                    out=d[0:1, :], in0=r2[0:1, 0:1], in1=r2[0:1, 1:2]
                )
                nc.vector.reciprocal(out=d[0:1, :], in_=d[0:1, :])
                # broadcast (-mn, rinv) to all partitions: build a 2x128 row
                # (ones row borrowed from sel) and DMA-repack to [128, 2]
                br = sm_pool.tile([1, 2 * P], F32, tag="br")
                brv = br[0:1, :].rearrange("o (p c) -> o p c", c=2)
                nc.vector.tensor_scalar_mul(
                    out=brv[:, :, 0:1], in0=sel[0:1, 0:P], scalar1=r2[0:1, 1:2]
                )
                nc.vector.tensor_scalar_mul(
                    out=brv[:, :, 1:2], in0=sel[0:1, 0:P], scalar1=d[0:1, 0:1]
                )
                bcol = sm_pool.tile([P, 2], F32, tag="bcol")
                nc.sync.dma_start(out=bcol[:, :], in_=brv)
                attn_t = sm_pool.tile([P, 32], F32, tag="attn")
                # attn = (score - mn) * 1/(mx - mn)
                nc.vector.tensor_scalar(
                    out=attn_t[:],
                    in0=score[:],
                    scalar1=bcol[:, 0:1],
                    scalar2=bcol[:, 1:2],
                    op0=mybir.AluOpType.add,
                    op1=mybir.AluOpType.mult,
                )
                nc.sync.dma_start(
                    out=attn_h[b].rearrange("(p f) -> p f", f=32), in_=attn_t[:]
                )
                sctx = sm_pool.tile([P, 32], F32, tag="sctx")
                nc.vector.tensor_mul(out=sctx[:], in0=attn_t[:], in1=vscale[:])
                # back to row-chunk layout: srow[n, c] = sctx at pos n*512+c
                srow = srow_pool.tile([NCHUNK, CW], F32, tag="srow")
                nc.sync.dma_start(
                    out=srow[:].rearrange("n (pl f) -> n pl f", f=32),
                    in_=sctx[:],
                )

                for n in range(NCHUNK):
                    bc = bc_pool.tile([P, CW], F32, tag="bc")
                    nc.tensor.matmul(
                        bc[:],
                        sel[:, n * P : (n + 1) * P],
                        srow[:, :],
                        start=True,
                        stop=True,
                    )
                    cols = slice(n * CW, (n + 1) * CW)
                    for cb in range(CB):
                        nc.vector.tensor_mul(
                            out=vts[cb][:, cols].bitcast(F32R),
                            in0=vts[cb][:, cols],
                            in1=bc[:],
                        )
                for cb in range(CB):
                    nc.sync.dma_start(
                        out=ctx_h[b, cb * P : (cb + 1) * P, :], in_=vts[cb][:]
                    )

    nc.compile()
    return nc


def get_nc():
    if "nc" not in _CACHE:
        _CACHE["nc"] = _build()
    return _CACHE["nc"]


def kernel(global_query, value, **run_kwargs):
    nc = get_nc()
    gq = np.ascontiguousarray(global_query, dtype=np.float32)
    v = np.ascontiguousarray(value, dtype=np.float32).reshape(B, D, WH)
    selc = np.kron(np.eye(NCHUNK), np.ones((1, P))).astype(np.float32)
    in_maps = []
    for i in range(NCORES):
        qs = gq[i * BL : (i + 1) * BL]  # [BL, D]
        # qlc[p, 2j] = 1.0, qlc[p, 2j+1] = q[b, cb*128+p] with j = b*CB+cb
        qlc = np.empty((P, 2 * BL * CB), np.float32)
        qlc[:, 0::2] = 1.0
        qlc[:, 1::2] = qs.reshape(BL * CB, P).T
        in_maps.append(
            {
                "v": v[i * BL : (i + 1) * BL],
                "qlc": qlc,
                "selc": selc,
            }
        )
    res = run_bass_kernel_spmd(nc, in_maps, core_ids=list(range(NCORES)), **run_kwargs)
    ctx = np.concatenate([np.asarray(r["ctx"]) for r in res.results], axis=0)
    attn = np.concatenate([np.asarray(r["attn"]) for r in res.results], axis=0)
    out_ctx = ctx.reshape(B, D, W, H).astype(np.float32)
    out_attn = attn.reshape(B, 1, W, H).astype(np.float32)
    _CACHE["last_results"] = res
    return out_ctx, out_attn


# revision 15
# speedup vs baseline: 1.0110x; 1.0110x over previous
"""CrossViewFusion kernel for Trainium2, 8 NeuronCores, batch-data-parallel.

Computes, per batch b:
    q   = l2norm(global_query[b])                 (cancels under min-max, skipped)
    vn  = value[b] / max(||value[b,:,w,h]||_2, eps)   (norm over channel dim)
    s   = einsum('d,dwh->wh', q, vn)
    attn= (s - min s) / (max s - min s)
    ctx = attn * vn
Returns (context [B,D,W,H], attn [B,1,W,H]) like the reference module.
"""

import sys

import numpy as np

if "/opt/trn_rl_repo" not in sys.path:
    sys.path.insert(0, "/opt/trn_rl_repo")

from contextlib import ExitStack

import concourse.bacc as bacc
import concourse.bass_isa as bass_isa
import concourse.mybir as mybir
import concourse.tile as tile
from concourse.bass_utils import run_bass_kernel_spmd

B, D, W, H = 32, 512, 64, 64
WH = W * H              # 4096
NCORES = 8
BL = B // NCORES        # 4 batches per core
P = 128
CB = D // P             # 4 channel blocks
QT = 4                  # quarters of the spatial dim per accumulation round
QW = WH // QT           # 1024
CW = 512                # matmul free-dim chunk (one PSUM bank of fp32)
NCHUNK = WH // CW       # 8
F32 = mybir.dt.float32
F32R = mybir.dt.float32r
EPS = 1e-12

_CACHE = {}


def _build():
    nc = bacc.Bacc("TRN2", target_bir_lowering=False, debug=False)

    v_h = nc.dram_tensor("v", [BL, D, WH], F32R, kind="ExternalInput")
    qlc_h = nc.dram_tensor("qlc", [P, 2 * BL * CB], F32R, kind="ExternalInput")
    selc_h = nc.dram_tensor("selc", [NCHUNK, NCHUNK * P], F32, kind="ExternalInput")
    ctx_h = nc.dram_tensor("ctx", [BL, D, WH], F32, kind="ExternalOutput")
    attn_h = nc.dram_tensor("attn", [BL, WH], F32, kind="ExternalOutput")

    with tile.TileContext(nc) as tc:
        with ExitStack() as ex:
            v_pool = ex.enter_context(tc.tile_pool(name="v", bufs=2 * CB))
            vsq_pool = ex.enter_context(tc.tile_pool(name="vsq", bufs=6))
            sums_pool = ex.enter_context(tc.tile_pool(name="sums", bufs=3))
            nm_pool = ex.enter_context(tc.tile_pool(name="nm", bufs=2))
            sm_pool = ex.enter_context(tc.tile_pool(name="sm", bufs=2))
            srow_pool = ex.enter_context(tc.tile_pool(name="srow", bufs=2))
            cpool = ex.enter_context(tc.tile_pool(name="const", bufs=1))
            ss_pool = ex.enter_context(
                tc.tile_pool(name="ss", bufs=2, space="PSUM")
            )
            sc_pool = ex.enter_context(
                tc.tile_pool(name="sc", bufs=2, space="PSUM")
            )
            bc_pool = ex.enter_context(
                tc.tile_pool(name="bc", bufs=2, space="PSUM")
            )

            # qL holds [ones | q_block] column pairs for every (batch, channel
            # block): col 2j = 1.0, col 2j+1 = q[b, cb*128:(cb+1)*128].
            qL = cpool.tile([P, 2 * BL * CB], F32, tag="qL")
            # sel[k, n*128:(n+1)*128] = (k == n): one-hot lhsT blocks used to
            # broadcast row n of an [8, 512] tile to all 128 partitions via PE.
            sel = cpool.tile([NCHUNK, NCHUNK * P], F32, tag="sel")
            nc.sync.dma_start(out=qL[:].bitcast(F32R), in_=qlc_h[:, :])
            nc.sync.dma_start(out=sel[:], in_=selc_h[:, :])

            for b in range(BL):
                vts = []
                for cb in range(CB):
                    vt = v_pool.tile([P, WH], F32, tag="v")
                    nc.sync.dma_start(
                        out=vt[:].bitcast(F32R),
                        in_=v_h[b, cb * P : (cb + 1) * P, :],
                    )
                    vts.append(vt)

                # nm[:, 0:32] = sumsq, nm[:, 32:64] = raw score, laid out as
                # pos = p*32 + f  (partition-major).
                nm = nm_pool.tile([P, 64], F32, tag="nm")
                for qtr in range(QT):
                    vsqs = []
                    for cb in range(CB):
                        vsq = vsq_pool.tile([P, QW], F32, tag="vsq")
                        nc.scalar.square(
                            out=vsq[:].bitcast(F32R),
                            in_=vts[cb][:, qtr * QW : (qtr + 1) * QW],
                        )
                        vsqs.append(vsq)
                    for h in range(QW // CW):
                        # global 512-chunk index; covers nm partitions
                        # [c*16, (c+1)*16) in the pos = p*32 + f layout
                        c = qtr * (QW // CW) + h
                        sl = slice(h * CW, (h + 1) * CW)
                        gsl = slice(qtr * QW + h * CW, qtr * QW + (h + 1) * CW)
                        ss = ss_pool.tile([1, CW], F32, tag="ss")
                        sc = sc_pool.tile([1, CW], F32, tag="sc")
                        for cb in range(CB):
                            j = b * CB + cb
                            # sumsq chain: lhsT = ones column
                            nc.tensor.matmul(
                                ss[0:1, :],
                                qL[:, 2 * j : 2 * j + 1].bitcast(F32R),
                                vsqs[cb][:, sl].bitcast(F32R),
                                start=(cb == 0),
                                stop=(cb == CB - 1),
                            )
                            # raw score chain: lhsT = q column
                            nc.tensor.matmul(
                                sc[0:1, :],
                                qL[:, 2 * j + 1 : 2 * j + 2].bitcast(F32R),
                                vts[cb][:, gsl].bitcast(F32R),
                                start=(cb == 0),
                                stop=(cb == CB - 1),
                            )
                        ss_sb = sums_pool.tile([1, CW], F32, tag="ss_sb")
                        sc_sb = sums_pool.tile([1, CW], F32, tag="sc_sb")
                        nc.scalar.copy(out=ss_sb[:], in_=ss[:])
                        nc.vector.tensor_copy(out=sc_sb[:], in_=sc[:])
                        psl = slice(c * 16, (c + 1) * 16)
                        nc.sync.dma_start(
                            out=nm[psl, 0:32],
                            in_=ss_sb[:].rearrange("o (p f) -> o p f", f=32),
                        )
                        nc.sync.dma_start(
                            out=nm[psl, 32:64],
                            in_=sc_sb[:].rearrange("o (p f) -> o p f", f=32),
                        )

                # per-position scalar chain in [128, 32] layout
                vscale = sm_pool.tile([P, 32], F32, tag="vscale")
                nc.scalar.sqrt(out=vscale[:], in_=nm[:, 0:32])
                nc.vector.tensor_scalar_max(
                    out=vscale[:], in0=vscale[:], scalar1=EPS
                )
                nc.vector.reciprocal(out=vscale[:], in_=vscale[:])
                score = sm_pool.tile([P, 32], F32, tag="score")
                nc.vector.tensor_mul(
                    out=score[:], in0=nm[:, 32:64], in1=vscale[:]
                )
                # per-partition max / -min pairs, then a tiny DMA repack to a
                # single row so DVE can finish the cross-partition reduction
                # (keeps gpsimd free for the cast-loads).
                mm2 = sm_pool.tile([P, 2], F32, tag="mm2")
                nc.vector.tensor_reduce(
                    out=mm2[:, 0:1],
                    in_=score[:],
                    axis=mybir.AxisListType.X,
                    op=mybir.AluOpType.max,
                )
                nc.vector.tensor_reduce(
                    out=mm2[:, 1:2],
                    in_=score[:],
                    axis=mybir.AxisListType.X,
                    op=mybir.AluOpType.min,
                )
                nc.vector.tensor_scalar_mul(
                    out=mm2[:, 1:2], in0=mm2[:, 1:2], scalar1=-1.0
                )
                r256 = sm_pool.tile([1, 2 * P], F32, tag="r256")
                nc.sync.dma_start(
                    out=r256[0:1, :].rearrange("o (p c) -> o p c", c=2),
                    in_=mm2[:, :],
                )
                r2 = sm_pool.tile([1, 2], F32, tag="r2")  # [mx, -mn]
                nc.vector.tensor_reduce(
                    out=r2[0:1, :],
                    in_=r256[0:1, :].rearrange("o (p c) -> o c p", c=2),
                    axis=mybir.AxisListType.X,
                    op=mybir.AluOpType.max,
                )
                d = sm_pool.tile([1, 1], F32, tag="d")  # 1/(mx - mn)
                nc.vector.tensor_add(
                    out=d[0:1, :], in0=r2[0:1, 0:1], in1=r2[0:1, 1:2]
                )
                nc.vector.reciprocal(out=d[0:1, :], in_=d[0:1, :])
                # broadcast (-mn, rinv) to all partitions: build a 2x128 row
                # (ones row borrowed from sel) and DMA-repack to [128, 2]
                br = sm_pool.tile([1, 2 * P], F32, tag="br")
                brv = br[0:1, :].rearrange("o (p c) -> o p c", c=2)
                nc.vector.tensor_scalar_mul(
                    out=brv[:, :, 0:1], in0=sel[0:1, 0:P], scalar1=r2[0:1, 1:2]
                )
                nc.vector.tensor_scalar_mul(
                    out=brv[:, :, 1:2], in0=sel[0:1, 0:P], scalar1=d[0:1, 0:1]
                )
                bcol = sm_pool.tile([P, 2], F32, tag="bcol")
                nc.sync.dma_start(out=bcol[:, :], in_=brv)
                attn_t = sm_pool.tile([P, 32], F32, tag="attn")
                # attn = (score - mn) * 1/(mx - mn)
                nc.vector.tensor_scalar(
                    out=attn_t[:],
                    in0=score[:],
                    scalar1=bcol[:, 0:1],
                    scalar2=bcol[:, 1:2],
                    op0=mybir.AluOpType.add,
                    op1=mybir.AluOpType.mult,
                )
                nc.sync.dma_start(
                    out=attn_h[b].rearrange("(p f) -> p f", f=32), in_=attn_t[:]
                )
                sctx = sm_pool.tile([P, 32], F32, tag="sctx")
                nc.vector.tensor_mul(out=sctx[:], in0=attn_t[:], in1=vscale[:])
                # back to row-chunk layout: srow[n, c] = sctx at pos n*512+c
                srow = srow_pool.tile([NCHUNK, CW], F32, tag="srow")
                nc.sync.dma_start(
                    out=srow[:].rearrange("n (pl f) -> n pl f", f=32),
                    in_=sctx[:],
                )

                for n in range(NCHUNK):
                    bc = bc_pool.tile([P, CW], F32, tag="bc")
                    nc.tensor.matmul(
                        bc[:],
                        sel[:, n * P : (n + 1) * P],
                        srow[:, :],
                        start=True,
                        stop=True,
                    )
                    cols = slice(n * CW, (n + 1) * CW)
                    for cb in range(CB):
                        nc.vector.tensor_mul(
                            out=vts[cb][:, cols].bitcast(F32R),
                            in0=vts[cb][:, cols],
                            in1=bc[:],
                        )
                for cb in range(CB):
                    nc.sync.dma_start(
                        out=ctx_h[b, cb * P : (cb + 1) * P, :], in_=vts[cb][:]
                    )

    nc.compile()
    return nc


def get_nc():
    if "nc" not in _CACHE:
        _CACHE["nc"] = _build()
    return _CACHE["nc"]


def kernel(global_query, value, **run_kwargs):
    nc = get_nc()
    gq = np.ascontiguousarray(global_query, dtype=np.float32)
    v = np.ascontiguousarray(value, dtype=np.float32).reshape(B, D, WH)
    selc = np.kron(np.eye(NCHUNK), np.ones((1, P))).astype(np.float32)
    in_maps = []
    for i in range(NCORES):
        qs = gq[i * BL : (i + 1) * BL]  # [BL, D]
        # qlc[p, 2j] = 1.0, qlc[p, 2j+1] = q[b, cb*128+p] with j = b*CB+cb
        qlc = np.empty((P, 2 * BL * CB), np.float32)
        qlc[:, 0::2] = 1.0
        qlc[:, 1::2] = qs.reshape(BL * CB, P).T
        in_maps.append(
            {
                "v": v[i * BL : (i + 1) * BL],
                "qlc": qlc,
                "selc": selc,
            }
        )
    res = run_bass_kernel_spmd(nc, in_maps, core_ids=list(range(NCORES)), **run_kwargs)
    ctx = np.concatenate([np.asarray(r["ctx"]) for r in res.results], axis=0)
    attn = np.concatenate([np.asarray(r["attn"]) for r in res.results], axis=0)
    out_ctx = ctx.reshape(B, D, W, H).astype(np.float32)
    out_attn = attn.reshape(B, 1, W, H).astype(np.float32)
    _CACHE["last_results"] = res
    return out_ctx, out_attn


# revision 16
# speedup vs baseline: 1.0378x; 1.0265x over previous
"""CrossViewFusion kernel for Trainium2, 8 NeuronCores, batch-data-parallel.

Computes, per batch b:
    q   = l2norm(global_query[b])                 (cancels under min-max, skipped)
    vn  = value[b] / max(||value[b,:,w,h]||_2, eps)   (norm over channel dim)
    s   = einsum('d,dwh->wh', q, vn)
    attn= (s - min s) / (max s - min s)
    ctx = attn * vn
Returns (context [B,D,W,H], attn [B,1,W,H]) like the reference module.
"""

import sys

import numpy as np

if "/opt/trn_rl_repo" not in sys.path:
    sys.path.insert(0, "/opt/trn_rl_repo")

from contextlib import ExitStack

import concourse.bacc as bacc
import concourse.bass_isa as bass_isa
import concourse.mybir as mybir
import concourse.tile as tile
from concourse.bass_utils import run_bass_kernel_spmd

B, D, W, H = 32, 512, 64, 64
WH = W * H              # 4096
NCORES = 8
BL = B // NCORES        # 4 batches per core
P = 128
CB = D // P             # 4 channel blocks
QT = 4                  # quarters of the spatial dim per accumulation round
QW = WH // QT           # 1024
CW = 512                # matmul free-dim chunk (one PSUM bank of fp32)
NCHUNK = WH // CW       # 8
F32 = mybir.dt.float32
F32R = mybir.dt.float32r
EPS = 1e-12

_CACHE = {}


def _build():
    nc = bacc.Bacc("TRN2", target_bir_lowering=False, debug=False)

    v_h = nc.dram_tensor("v", [BL, D, WH], F32R, kind="ExternalInput")
    qlc_h = nc.dram_tensor("qlc", [P, 2 * BL * CB], F32R, kind="ExternalInput")
    selc_h = nc.dram_tensor("selc", [NCHUNK, NCHUNK * P], F32R, kind="ExternalInput")
    ctx_h = nc.dram_tensor("ctx", [BL, D, WH], F32, kind="ExternalOutput")
    attn_h = nc.dram_tensor("attn", [BL, WH], F32, kind="ExternalOutput")

    with tile.TileContext(nc) as tc:
        with ExitStack() as ex:
            v_pool = ex.enter_context(tc.tile_pool(name="v", bufs=2 * CB))
            vsq_pool = ex.enter_context(tc.tile_pool(name="vsq", bufs=6))
            sums_pool = ex.enter_context(tc.tile_pool(name="sums", bufs=3))
            nm_pool = ex.enter_context(tc.tile_pool(name="nm", bufs=2))
            sm_pool = ex.enter_context(tc.tile_pool(name="sm", bufs=2))
            srow_pool = ex.enter_context(tc.tile_pool(name="srow", bufs=2))
            cpool = ex.enter_context(tc.tile_pool(name="const", bufs=1))
            ss_pool = ex.enter_context(
                tc.tile_pool(name="ss", bufs=2, space="PSUM")
            )
            sc_pool = ex.enter_context(
                tc.tile_pool(name="sc", bufs=2, space="PSUM")
            )
            bc_pool = ex.enter_context(
                tc.tile_pool(name="bc", bufs=2, space="PSUM")
            )

            # qL holds [ones | q_block] column pairs for every (batch, channel
            # block): col 2j = 1.0, col 2j+1 = q[b, cb*128:(cb+1)*128].
            qL = cpool.tile([P, 2 * BL * CB], F32, tag="qL")
            # sel[k, n*128:(n+1)*128] = (k == n): one-hot lhsT blocks used to
            # broadcast row n of an [8, 512] tile to all 128 partitions via PE.
            sel = cpool.tile([NCHUNK, NCHUNK * P], F32, tag="sel")
            nc.sync.dma_start(out=qL[:].bitcast(F32R), in_=qlc_h[:, :])
            nc.sync.dma_start(out=sel[:].bitcast(F32R), in_=selc_h[:, :])

            def emit_loads(b):
                tiles = []
                for cb in range(CB):
                    vt = v_pool.tile([P, WH], F32, tag="v")
                    nc.sync.dma_start(
                        out=vt[:].bitcast(F32R),
                        in_=v_h[b, cb * P : (cb + 1) * P, :],
                    )
                    tiles.append(vt)
                return tiles

            # software-pipelined emission: keep the sync-engine stream free of
            # blocking small DMAs ahead of the next batch's big loads
            vts_by_b = {0: emit_loads(0)}
            if BL > 1:
                vts_by_b[1] = emit_loads(1)

            for b in range(BL):
                vts = vts_by_b.pop(b)

                # nm[:, 0:32] = sumsq, nm[:, 32:64] = raw score, laid out as
                # pos = p*32 + f  (partition-major).
                nm = nm_pool.tile([P, 64], F32, tag="nm")
                for qtr in range(QT):
                    vsqs = []
                    for cb in range(CB):
                        vsq = vsq_pool.tile([P, QW], F32, tag="vsq")
                        nc.scalar.square(
                            out=vsq[:].bitcast(F32R),
                            in_=vts[cb][:, qtr * QW : (qtr + 1) * QW],
                        )
                        vsqs.append(vsq)
                    for h in range(QW // CW):
                        # global 512-chunk index; covers nm partitions
                        # [c*16, (c+1)*16) in the pos = p*32 + f layout
                        c = qtr * (QW // CW) + h
                        sl = slice(h * CW, (h + 1) * CW)
                        gsl = slice(qtr * QW + h * CW, qtr * QW + (h + 1) * CW)
                        ss = ss_pool.tile([1, CW], F32, tag="ss")
                        sc = sc_pool.tile([1, CW], F32, tag="sc")
                        for cb in range(CB):
                            j = b * CB + cb
                            # sumsq chain: lhsT = ones column
                            nc.tensor.matmul(
                                ss[0:1, :],
                                qL[:, 2 * j : 2 * j + 1].bitcast(F32R),
                                vsqs[cb][:, sl].bitcast(F32R),
                                start=(cb == 0),
                                stop=(cb == CB - 1),
                            )
                            # raw score chain: lhsT = q column
                            nc.tensor.matmul(
                                sc[0:1, :],
                                qL[:, 2 * j + 1 : 2 * j + 2].bitcast(F32R),
                                vts[cb][:, gsl].bitcast(F32R),
                                start=(cb == 0),
                                stop=(cb == CB - 1),
                            )
                        ss_sb = sums_pool.tile([1, CW], F32, tag="ss_sb")
                        sc_sb = sums_pool.tile([1, CW], F32, tag="sc_sb")
                        nc.scalar.copy(out=ss_sb[:], in_=ss[:])
                        nc.vector.tensor_copy(out=sc_sb[:], in_=sc[:])
                        psl = slice(c * 16, (c + 1) * 16)
                        nc.sync.dma_start(
                            out=nm[psl, 0:32],
                            in_=ss_sb[:].rearrange("o (p f) -> o p f", f=32),
                        )
                        nc.sync.dma_start(
                            out=nm[psl, 32:64],
                            in_=sc_sb[:].rearrange("o (p f) -> o p f", f=32),
                        )

                # per-position scalar chain in [128, 32] layout
                vscale = sm_pool.tile([P, 32], F32, tag="vscale")
                nc.scalar.sqrt(out=vscale[:], in_=nm[:, 0:32])
                nc.vector.tensor_scalar_max(
                    out=vscale[:], in0=vscale[:], scalar1=EPS
                )
                nc.vector.reciprocal(out=vscale[:], in_=vscale[:])
                score = sm_pool.tile([P, 32], F32, tag="score")
                nc.vector.tensor_mul(
                    out=score[:], in0=nm[:, 32:64], in1=vscale[:]
                )
                # per-partition max / -min pairs, then a tiny DMA repack to a
                # single row so DVE can finish the cross-partition reduction
                # (keeps gpsimd free for the cast-loads).
                mm2 = sm_pool.tile([P, 2], F32, tag="mm2")
                nc.vector.tensor_reduce(
                    out=mm2[:, 0:1],
                    in_=score[:],
                    axis=mybir.AxisListType.X,
                    op=mybir.AluOpType.max,
                )
                nc.vector.tensor_reduce(
                    out=mm2[:, 1:2],
                    in_=score[:],
                    axis=mybir.AxisListType.X,
                    op=mybir.AluOpType.min,
                )
                nc.vector.tensor_scalar_mul(
                    out=mm2[:, 1:2], in0=mm2[:, 1:2], scalar1=-1.0
                )
                r256 = sm_pool.tile([1, 2 * P], F32, tag="r256")
                nc.sync.dma_start(
                    out=r256[0:1, :].rearrange("o (p c) -> o p c", c=2),
                    in_=mm2[:, :],
                )
                r2 = sm_pool.tile([1, 2], F32, tag="r2")  # [mx, -mn]
                nc.vector.tensor_reduce(
                    out=r2[0:1, :],
                    in_=r256[0:1, :].rearrange("o (p c) -> o c p", c=2),
                    axis=mybir.AxisListType.X,
                    op=mybir.AluOpType.max,
                )
                d = sm_pool.tile([1, 1], F32, tag="d")  # 1/(mx - mn)
                nc.vector.tensor_add(
                    out=d[0:1, :], in0=r2[0:1, 0:1], in1=r2[0:1, 1:2]
                )
                nc.vector.reciprocal(out=d[0:1, :], in_=d[0:1, :])
                # broadcast (-mn, rinv) to all partitions: build a 2x128 row
                # (ones row borrowed from sel) and DMA-repack to [128, 2]
                br = sm_pool.tile([1, 2 * P], F32, tag="br")
                brv = br[0:1, :].rearrange("o (p c) -> o p c", c=2)
                nc.vector.tensor_scalar_mul(
                    out=brv[:, :, 0:1], in0=sel[0:1, 0:P], scalar1=r2[0:1, 1:2]
                )
                nc.vector.tensor_scalar_mul(
                    out=brv[:, :, 1:2], in0=sel[0:1, 0:P], scalar1=d[0:1, 0:1]
                )
                bcol = sm_pool.tile([P, 2], F32, tag="bcol")
                nc.sync.dma_start(out=bcol[:, :], in_=brv)
                attn_t = sm_pool.tile([P, 32], F32, tag="attn")
                # attn = (score - mn) * 1/(mx - mn)
                nc.vector.tensor_scalar(
                    out=attn_t[:],
                    in0=score[:],
                    scalar1=bcol[:, 0:1],
                    scalar2=bcol[:, 1:2],
                    op0=mybir.AluOpType.add,
                    op1=mybir.AluOpType.mult,
                )
                nc.sync.dma_start(
                    out=attn_h[b].rearrange("(p f) -> p f", f=32), in_=attn_t[:]
                )
                sctx = sm_pool.tile([P, 32], F32, tag="sctx")
                nc.vector.tensor_mul(
                    out=sctx[:].bitcast(F32R), in0=attn_t[:], in1=vscale[:]
                )
                # back to row-chunk layout: srow[n, c] = sctx at pos n*512+c
                srow = srow_pool.tile([NCHUNK, CW], F32, tag="srow")
                nc.sync.dma_start(
                    out=srow[:].rearrange("n (pl f) -> n pl f", f=32).bitcast(F32R),
                    in_=sctx[:].bitcast(F32R),
                )

                for n in range(NCHUNK):
                    bc = bc_pool.tile([P, CW], F32, tag="bc")
                    nc.tensor.matmul(
                        bc[:],
                        sel[:, n * P : (n + 1) * P].bitcast(F32R),
                        srow[:, :].bitcast(F32R),
                        start=True,
                        stop=True,
                    )
                    cols = slice(n * CW, (n + 1) * CW)
                    for cb in range(CB):
                        nc.vector.tensor_mul(
                            out=vts[cb][:, cols].bitcast(F32R),
                            in0=vts[cb][:, cols],
                            in1=bc[:],
                        )
                for cb in range(CB):
                    nc.sync.dma_start(
                        out=ctx_h[b, cb * P : (cb + 1) * P, :], in_=vts[cb][:]
                    )
                if b + 2 < BL:
                    vts_by_b[b + 2] = emit_loads(b + 2)

    nc.compile()
    return nc


def get_nc():
    if "nc" not in _CACHE:
        _CACHE["nc"] = _build()
    return _CACHE["nc"]


def kernel(global_query, value, **run_kwargs):
    nc = get_nc()
    gq = np.ascontiguousarray(global_query, dtype=np.float32)
    v = np.ascontiguousarray(value, dtype=np.float32).reshape(B, D, WH)
    selc = np.kron(np.eye(NCHUNK), np.ones((1, P))).astype(np.float32)
    in_maps = []
    for i in range(NCORES):
        qs = gq[i * BL : (i + 1) * BL]  # [BL, D]
        # qlc[p, 2j] = 1.0, qlc[p, 2j+1] = q[b, cb*128+p] with j = b*CB+cb
        qlc = np.empty((P, 2 * BL * CB), np.float32)
        qlc[:, 0::2] = 1.0
        qlc[:, 1::2] = qs.reshape(BL * CB, P).T
        in_maps.append(
            {
                "v": v[i * BL : (i + 1) * BL],
                "qlc": qlc,
                "selc": selc,
            }
        )
    res = run_bass_kernel_spmd(nc, in_maps, core_ids=list(range(NCORES)), **run_kwargs)
    ctx = np.concatenate([np.asarray(r["ctx"]) for r in res.results], axis=0)
    attn = np.concatenate([np.asarray(r["attn"]) for r in res.results], axis=0)
    out_ctx = ctx.reshape(B, D, W, H).astype(np.float32)
    out_attn = attn.reshape(B, 1, W, H).astype(np.float32)
    _CACHE["last_results"] = res
    return out_ctx, out_attn
